# revision 12
# baseline (speedup 1.0000x reference)
"""BiDAF2 attention kernel for Trainium2, 8-core data parallel over batch.

reference (per batch b):
  w1h[s,l] = h[s,:] @ w1_w[l,:] + w1_b[l]
  w2q[t,l] = q[t,:] @ w2_w[l,:] + w2_b[l]
  a[s,t]   = w1h[s,t] + w2q[t,s] + h[s,:]@q[t,:]
  p        = softmax_t(a);  c[s,:] = p[s,:] @ q
  m[s]     = max_t a[s,t];  p2 = softmax_s(m)
  out      = concat([h, c, h*c, (h*p2)*c], axis=-1)

v2 strategy per core (2 batches):
  - Algebra: a[s,t] = (h[s]+w2_w[s])@q[t] + h[s]@w1_w[t] + w1_b[t] + w2_b[s].
    The w1 moving operand (w1T) is batch-independent: transposed once per
    core. No per-batch u=q+w1 add/cast chain. w2_b cancels in softmax_t and
    is only added back to the row max before the p2 softmax.
  - fp16 everywhere on the PE (fp32 PSUM accumulation). 3 full passes per
    batch: (h+w2)@qT, h@w1T (both into the logits PSUM), p@q for c.
  - Loads are single big gpsimd (SWDGE) DMAs casting f32->f16 in flight.
  - Transposes are one-shot xbar DMAs into [p][chunk][k][128] tiles whose
    flat free layout matches the xbar tile stream (48 blocks of 128x128).
  - All cross-batch pools are double-buffered so batch b+1 prep overlaps
    batch b phase A/B. DMA issue is spread over SP/ACT/Pool queues.
  - a lives only in PSUM; row max via DVE reduce_max(negate); exp on ACT with
    fused row-sum; p written fp16; p transposed per s-tile; c accumulates
    over 8 t-chunks in PSUM; 1/Z folded into the c epilogue scale.
  - p2 (softmax over the 1024 row maxes) via 4KB DRAM rearrange to one
    partition row, softmaxed, scattered back; applied on ACT in the epilogue.
"""

import os
import sys

for _p in ("/opt/trn_rl_repo", "/root/.axon_site/_ro/trn_rl_repo"):
    if os.path.isdir(_p) and _p not in sys.path:
        sys.path.append(_p)

from contextlib import ExitStack

import numpy as np

import concourse.bass as bass
import concourse.tile as tile
from concourse import bacc, mybir
from concourse.bass_utils import run_bass_kernel_spmd

B, L, D = 16, 1024, 768
NCORES = 8
BL = B // NCORES  # batches per core
P = 128
KD = D // P  # 6 d-chunks
NT = L // P  # 8 t-chunks == 8 s-tiles
F16 = mybir.dt.float16
F32 = mybir.dt.float32
EXP = mybir.ActivationFunctionType.Exp
COPY = mybir.ActivationFunctionType.Copy
AX = mybir.AxisListType.X

REPEAT = 1  # benchmarking aid: run the whole body REPEAT times via For_i
PYREPEAT = 1  # python-unrolled repeat (for TimelineSim steady-state estimates)


def _make_pools(ctx: ExitStack, tc: tile.TileContext):
    return dict(
        statics=ctx.enter_context(tc.tile_pool(name="statics", bufs=1)),
        bigs=ctx.enter_context(tc.tile_pool(name="bigs", bufs=2)),
        scr=ctx.enter_context(tc.tile_pool(name="scr", bufs=2)),
        smalls=ctx.enter_context(tc.tile_pool(name="smalls", bufs=2)),
        dram=ctx.enter_context(tc.tile_pool(name="dram", bufs=2, space="DRAM")),
        psA=ctx.enter_context(tc.tile_pool(name="psA", bufs=2, space="PSUM")),
        psC=ctx.enter_context(tc.tile_pool(name="psC", bufs=2, space="PSUM")),
    )


def _emit(ctx: ExitStack, tc: tile.TileContext, h, q, w1w, w1b, w2w, w2b, out):
    pools = _make_pools(ctx, tc)
    if REPEAT > 1:
        with tc.For_i(0, REPEAT, 1):
            _emit_once(pools, tc, h, q, w1w, w1b, w2w, w2b, out)
    else:
        for _ in range(PYREPEAT):
            _emit_once(pools, tc, h, q, w1w, w1b, w2w, w2b, out)


def _emit_once(pools, tc: tile.TileContext, h, q, w1w, w1b, w2w, w2b, out):
    nc = tc.nc
    halves = [(0, 512), (512, 1024)]

    statics = pools["statics"]
    bigs = pools["bigs"]
    scr = pools["scr"]
    smalls = pools["smalls"]
    dram = pools["dram"]
    psA = pools["psA"]
    psC = pools["psC"]

    # ---- statics (once per core) ----
    ones1 = statics.tile([1, P], F16, tag="ones1")
    nc.vector.memset(ones1, 1.0)
    w1b16 = statics.tile([1, L], F16, tag="w1b16")
    nc.gpsimd.dma_start(out=w1b16, in_=w1b[None, :])
    w2b_col = statics.tile([P, NT], F32, tag="w2b_col")
    nc.sync.dma_start(out=w2b_col, in_=w2b.rearrange("(c p) -> p c", p=P))

    # ---- prep for both batches, emitted up front so loads(b1) are queued
    # ahead of outs(b0) on the Pool queue and transposes(b1) ahead of the
    # phase-A pT transposes on SP. Load order tuned for the critical path
    # into phase A tile 0: q16(0) -> w2n16 -> h16(0) -> w1n16.
    prep = []
    w2n16 = statics.tile([P, NT, D], F16, tag="w2n16")
    w1T4 = statics.tile([P, NT, KD, P], F16, tag="w1T4")
    for b in range(BL):
        q16 = bigs.tile([P, NT, D], F16, tag="q16")
        nc.gpsimd.dma_start(out=q16, in_=q[b].rearrange("(c p) d -> p c d", p=P))
        h16 = bigs.tile([P, NT, D], F16, tag="h16")
        nc.gpsimd.dma_start(out=h16, in_=h[b].rearrange("(c p) d -> p c d", p=P))
        if b == 0:
            nc.gpsimd.dma_start(
                out=w2n16, in_=w2w.rearrange("(c p) d -> p c d", p=P)
            )
            # w1n16 borrows the single-buffered "hw2" ring slot (dead after
            # the w1T4 transpose; the first hw2 add recycles it via WAR).
            w1n16 = bigs.tile([P, NT, D], F16, tag="hw2", bufs=1)
            nc.gpsimd.dma_start(
                out=w1n16, in_=w1w.rearrange("(c p) d -> p c d", p=P)
            )
        hw2 = bigs.tile([P, NT, D], F16, tag="hw2", bufs=1)
        # b0's add is on the critical path into phase A: DVE (fast, and the
        # DVE FIFO is empty this early). b1's add runs in A(0)'s shadow on
        # Pool so it cannot block A(0)'s reduce_max stream on DVE.
        if b == 0:
            nc.vector.tensor_add(hw2, h16, w2n16)
        else:
            nc.gpsimd.tensor_add(out=hw2, in0=h16, in1=w2n16)

        qT4 = bigs.tile([P, NT, KD, P], F16, tag="qT4")
        nc.sync.dma_start(
            out=qT4.rearrange("p c k r -> p (c k) r"),
            in_=q16.rearrange("p c d -> p (c d)"),
            transpose=True,
        )
        if b == 0:
            nc.sync.dma_start(
                out=w1T4.rearrange("p c k r -> p (c k) r"),
                in_=w1n16.rearrange("p c d -> p (c d)"),
                transpose=True,
            )
        hT4 = bigs.tile([P, NT, KD, P], F16, tag="hT4")
        nc.sync.dma_start(
            out=hT4.rearrange("p c k r -> p (c k) r"),
            in_=h16.rearrange("p c d -> p (c d)"),
            transpose=True,
        )
        hw2T4 = bigs.tile([P, NT, KD, P], F16, tag="hw2T4")
        nc.sync.dma_start(
            out=hw2T4.rearrange("p c k r -> p (c k) r"),
            in_=hw2.rearrange("p c d -> p (c d)"),
            transpose=True,
        )
        prep.append((q16, h16, qT4, hT4, hw2T4))

    for b in range(BL):
        q16, h16, qT4, hT4, hw2T4 = prep[b]
        pT_all = bigs.tile([P, NT, L], F16, tag="pT", bufs=1)
        m_negcol = smalls.tile([P, NT], F32, tag="m_negcol")
        z_col = smalls.tile([P, NT], F32, tag="z_col")
        r_col = smalls.tile([P, NT], F32, tag="r_col")

        # ---- phase A: logits + softmax_t per s-tile ----
        for i in range(NT):
            s0 = i * P
            ps_a = psA.tile([P, L], F32)
            for t0, t1 in halves:
                nc.tensor.matmul(
                    ps_a[:, t0:t1], ones1, w1b16[:, t0:t1], start=True, stop=False
                )
            # (h+w2)@qT pass first: its operands are ready before w1T4/hT4
            for k in range(KD):
                st_hw2 = hw2T4[:, i, k, :]
                for hf in range(2):
                    nc.tensor.matmul(
                        ps_a[:, hf * 512:(hf + 1) * 512], st_hw2,
                        qT4[:, hf * 4:(hf + 1) * 4, k, :],
                        start=False, stop=False,
                    )
            for k in range(KD):
                st_h = hT4[:, i, k, :]
                last = k == KD - 1
                for hf in range(2):
                    nc.tensor.matmul(
                        ps_a[:, hf * 512:(hf + 1) * 512], st_h,
                        w1T4[:, hf * 4:(hf + 1) * 4, k, :],
                        start=False, stop=last,
                    )
            negm = m_negcol[:, i:i + 1]
            nc.vector.reduce_max(negm, ps_a, axis=AX, negate=True)
            p16 = scr.tile([P, L], F16, tag="p16")
            nc.scalar.activation(out=p16, in_=ps_a, func=EXP, bias=negm,
                                 scale=1.0, accum_out=z_col[:, i:i + 1])
            nc.scalar.dma_start(out=pT_all[:, :, s0:s0 + P], in_=p16,
                                transpose=True)

        # ---- p2 = softmax over all 1024 row maxes (depends on phase A only) ----
        m_true = smalls.tile([P, NT], F32, tag="m_true")
        nc.vector.tensor_sub(m_true, w2b_col, m_negcol)
        m_dram = dram.tile([L], F32, tag="m_dram")
        nc.sync.dma_start(out=m_dram.rearrange("(c p) -> p c", p=P), in_=m_true)
        m_row = smalls.tile([1, L], F32, tag="row_a", bufs=1)
        nc.sync.dma_start(out=m_row, in_=m_dram[None, :])
        negmm = smalls.tile([1, 1], F32, tag="negmm")
        nc.vector.reduce_max(negmm, m_row, axis=AX, negate=True)
        z2 = smalls.tile([1, 1], F32, tag="z2")
        e2 = smalls.tile([1, L], F32, tag="e2", bufs=1)
        nc.scalar.activation(out=e2, in_=m_row, func=EXP, bias=negmm,
                             scale=1.0, accum_out=z2)
        r2 = smalls.tile([1, 1], F32, tag="r2")
        nc.vector.reciprocal(r2, z2)
        p2_row = smalls.tile([1, L], F32, tag="row_a", bufs=1)
        nc.vector.tensor_scalar_mul(p2_row, in0=e2, scalar1=r2)
        p2_dram = dram.tile([L], F32, tag="p2_dram")
        nc.sync.dma_start(out=p2_dram[None, :], in_=p2_row)
        p2_col = smalls.tile([P, NT], F32, tag="p2_col")
        nc.sync.dma_start(out=p2_col, in_=p2_dram.rearrange("(c p) -> p c", p=P))

        # ---- phase B: c = p@q, epilogue ----
        for i in range(NT):
            s0 = i * P
            ps_c = psC.tile([P, D], F32)
            for tcn in range(NT):
                lp = pT_all[:, tcn, s0:s0 + P]
                nc.tensor.matmul(ps_c[:, 0:512], lp, q16[:, tcn, 0:512],
                                 start=(tcn == 0), stop=(tcn == NT - 1))
                nc.tensor.matmul(ps_c[:, 512:D], lp, q16[:, tcn, 512:D],
                                 start=(tcn == 0), stop=(tcn == NT - 1))
            r_i = r_col[:, i:i + 1]
            nc.vector.reciprocal(r_i, z_col[:, i:i + 1])
            # section 0 (h) straight from h16 via casting SWDGE store;
            # sections 1-3 (c, h*c, qcc) assembled -> one 9KB-row DMA
            nc.gpsimd.dma_start(out=out[b, s0:s0 + P, 0:D], in_=h16[:, i, :])
            osec = scr.tile([P, 3, D], F32, tag="osec")
            nc.vector.tensor_scalar_mul(osec[:, 0, :], in0=ps_c, scalar1=r_i)
            nc.vector.tensor_mul(osec[:, 1, :], h16[:, i, :], osec[:, 0, :])
            nc.scalar.activation(out=osec[:, 2, :], in_=osec[:, 1, :], func=COPY,
                                 scale=p2_col[:, i:i + 1])
            nc.gpsimd.dma_start(out=out[b, s0:s0 + P, D:], in_=osec)


def build():
    nc = bacc.Bacc()
    h = nc.dram_tensor("h", [BL, L, D], F32, kind="ExternalInput")
    q = nc.dram_tensor("q", [BL, L, D], F32, kind="ExternalInput")
    w1w = nc.dram_tensor("w1_w", [L, D], F32, kind="ExternalInput")
    w1b = nc.dram_tensor("w1_b", [L], F32, kind="ExternalInput")
    w2w = nc.dram_tensor("w2_w", [L, D], F32, kind="ExternalInput")
    w2b = nc.dram_tensor("w2_b", [L], F32, kind="ExternalInput")
    out = nc.dram_tensor("out", [BL, L, 4 * D], F32, kind="ExternalOutput")
    with tile.TileContext(nc) as tc, ExitStack() as ctx:
        _emit(ctx, tc, h[:], q[:], w1w[:], w1b[:], w2w[:], w2b[:], out[:])
    nc.compile()
    return nc


def _in_maps(inputs):
    arr = {k: np.ascontiguousarray(np.asarray(v, np.float32))
           for k, v in inputs.items()}
    maps = []
    for c in range(NCORES):
        sl = slice(c * BL, (c + 1) * BL)
        maps.append({
            "h": arr["h"][sl], "q": arr["q"][sl],
            "w1_w": arr["w1_w"], "w1_b": arr["w1_b"],
            "w2_w": arr["w2_w"], "w2_b": arr["w2_b"],
        })
    return maps


def kernel(**inputs):
    nc = build()
    res = run_bass_kernel_spmd(nc, _in_maps(inputs), core_ids=list(range(NCORES)))
    return np.concatenate([r["out"] for r in res.results], axis=0)


def run_profiled(inputs, **kwargs):
    nc = build()
    res = run_bass_kernel_spmd(
        nc, _in_maps(inputs), core_ids=list(range(NCORES)), trace=True, **kwargs
    )
    out = np.concatenate([r["out"] for r in res.results], axis=0)
    return out, res


# revision 34
# speedup vs baseline: 1.6378x; 1.6378x over previous
"""BiDAF2 attention kernel for Trainium2, 8-core data parallel over batch.

reference (per batch b):
  w1h[s,l] = h[s,:] @ w1_w[l,:] + w1_b[l]
  w2q[t,l] = q[t,:] @ w2_w[l,:] + w2_b[l]
  a[s,t]   = w1h[s,t] + w2q[t,s] + h[s,:]@q[t,:]
  p        = softmax_t(a);  c[s,:] = p[s,:] @ q
  m[s]     = max_t a[s,t];  p2 = softmax_s(m)
  out      = concat([h, c, h*c, (h*p2)*c], axis=-1)

v2 strategy per core (2 batches):
  - Algebra: a[s,t] = (h[s]+w2_w[s])@q[t] + h[s]@w1_w[t] + w1_b[t] + w2_b[s].
    The w1 moving operand (w1T) is batch-independent: transposed once per
    core. No per-batch u=q+w1 add/cast chain. w2_b cancels in softmax_t and
    is only added back to the row max before the p2 softmax.
  - fp16 everywhere on the PE (fp32 PSUM accumulation). 3 full passes per
    batch: (h+w2)@qT, h@w1T (both into the logits PSUM), p@q for c.
  - Loads are single big gpsimd (SWDGE) DMAs casting f32->f16 in flight.
  - Transposes are one-shot xbar DMAs into [p][chunk][k][128] tiles whose
    flat free layout matches the xbar tile stream (48 blocks of 128x128).
  - All cross-batch pools are double-buffered so batch b+1 prep overlaps
    batch b phase A/B. DMA issue is spread over SP/ACT/Pool queues.
  - a lives only in PSUM; row max via DVE reduce_max(negate); exp on ACT with
    fused row-sum; p written fp16; p transposed per s-tile; c accumulates
    over 8 t-chunks in PSUM; 1/Z folded into the c epilogue scale.
  - p2 (softmax over the 1024 row maxes) via 4KB DRAM rearrange to one
    partition row, softmaxed, scattered back; applied on ACT in the epilogue.
"""

import os
import sys

for _p in ("/opt/trn_rl_repo", "/root/.axon_site/_ro/trn_rl_repo"):
    if os.path.isdir(_p) and _p not in sys.path:
        sys.path.append(_p)

from contextlib import ExitStack

import numpy as np

import concourse.bass as bass
import concourse.tile as tile
from concourse import bacc, mybir
from concourse.bass_utils import run_bass_kernel_spmd

B, L, D = 16, 1024, 768
NCORES = 8
BL = B // NCORES  # batches per core
P = 128
KD = D // P  # 6 d-chunks
NT = L // P  # 8 t-chunks == 8 s-tiles
F16 = mybir.dt.float16
F32 = mybir.dt.float32
EXP = mybir.ActivationFunctionType.Exp
COPY = mybir.ActivationFunctionType.Copy
AX = mybir.AxisListType.X

REPEAT = 1  # benchmarking aid: run the whole body REPEAT times via For_i
PYREPEAT = 1  # python-unrolled repeat (for TimelineSim steady-state estimates)
# differential-bench switches (bench-only; grading always runs everything)
SKIP_OUT = False    # drop output DMAs (osec + h-section)
SKIP_B = False      # drop phase B entirely (c matmuls + epilogue)
SKIP_A_MM = False   # drop phase A logit matmuls (keeps bias so exp is sane)


def _make_pools(ctx: ExitStack, tc: tile.TileContext):
    return dict(
        statics=ctx.enter_context(tc.tile_pool(name="statics", bufs=1)),
        bigs=ctx.enter_context(tc.tile_pool(name="bigs", bufs=2)),
        scr=ctx.enter_context(tc.tile_pool(name="scr", bufs=2)),
        smalls=ctx.enter_context(tc.tile_pool(name="smalls", bufs=2)),
        dram=ctx.enter_context(tc.tile_pool(name="dram", bufs=2, space="DRAM")),
        psA=ctx.enter_context(tc.tile_pool(name="psA", bufs=2, space="PSUM")),
        psC=ctx.enter_context(tc.tile_pool(name="psC", bufs=2, space="PSUM")),
    )


def _emit(ctx: ExitStack, tc: tile.TileContext, h, q, w1w, w1b, w2w, w2b, out):
    pools = _make_pools(ctx, tc)
    st = _emit_statics(pools, tc, w1w, w1b, w2w, w2b)
    if REPEAT > 1:
        # For_i around a PYREPEAT-unrolled body: benching with PYREPEAT>1
        # amortizes any loop-boundary serialization over PYREPEAT iters
        with tc.For_i(0, REPEAT, 1):
            for _ in range(PYREPEAT):
                _emit_iter(pools, tc, st, h, q, out)
    else:
        for _ in range(PYREPEAT):
            _emit_iter(pools, tc, st, h, q, out)


def _emit_statics(pools, tc, w1w, w1b, w2w, w2b):
    """Per-core constants: loaded once, outside the repeat body."""
    nc = tc.nc
    statics, bigs = pools["statics"], pools["bigs"]
    ones1 = statics.tile([1, P], F16, tag="ones1")
    nc.vector.memset(ones1, 1.0)
    w1b16 = statics.tile([1, L], F16, tag="w1b16")
    nc.gpsimd.dma_start(out=w1b16, in_=w1b[None, :])
    w2b_col = statics.tile([P, NT], F32, tag="w2b_col")
    nc.sync.dma_start(out=w2b_col, in_=w2b.rearrange("(c p) -> p c", p=P))
    w2n16 = statics.tile([P, NT, D], F16, tag="w2n16")
    nc.gpsimd.dma_start(out=w2n16, in_=w2w.rearrange("(c p) d -> p c d", p=P))
    # w1n16 borrows the single-buffered "hw2" ring slot in bigs (dead after
    # the w1T4 transpose; the first hw2 add then recycles it via WAR).
    w1n16 = bigs.tile([P, NT, D], F16, tag="hw2", bufs=1)
    nc.gpsimd.dma_start(out=w1n16, in_=w1w.rearrange("(c p) d -> p c d", p=P))
    w1T4 = statics.tile([P, NT, KD, P], F16, tag="w1T4")
    nc.scalar.dma_start(
        out=w1T4.rearrange("p c k r -> p (c k) r"),
        in_=w1n16.rearrange("p c d -> p (c d)"),
        transpose=True,
    )
    return dict(ones1=ones1, w1b16=w1b16, w2b_col=w2b_col, w2n16=w2n16,
                w1T4=w1T4)


def _emit_iter(pools, tc, cst, h, q, out):
    nc = tc.nc
    bigs, scr, smalls = pools["bigs"], pools["scr"], pools["smalls"]
    dram, psA, psC = pools["dram"], pools["psA"], pools["psC"]
    ones1, w1b16 = cst["ones1"], cst["w1b16"]
    w2b_col, w2n16, w1T4 = cst["w2b_col"], cst["w2n16"], cst["w1T4"]

    # ---- prep for both batches, emitted up front so loads(b1) are queued
    # ahead of outs(b0) on the Pool queue. Loads are split in halves so a
    # single load never fills the 1024-entry SWDGE descriptor ring.
    bst = []
    for b in range(BL):
        q16 = bigs.tile([P, NT, D], F16, tag="q16")
        h16 = bigs.tile([P, NT, D], F16, tag="h16")
        nc.gpsimd.dma_start(out=q16, in_=q[b].rearrange("(c p) d -> p c d", p=P))
        nc.gpsimd.dma_start(out=h16, in_=h[b].rearrange("(c p) d -> p c d", p=P))
        hw2 = bigs.tile([P, NT, D], F16, tag="hw2", bufs=1)
        # b0's add is on the critical path into phase A: DVE (fast, and the
        # DVE FIFO is empty this early). b1's add runs in A(0)'s shadow on
        # Pool so it cannot block A(0)'s reduce_max stream on DVE.
        nc.vector.tensor_add(hw2, h16, w2n16)

        qT4 = bigs.tile([P, NT, KD, P], F16, tag="qT4")
        nc.scalar.dma_start(
            out=qT4.rearrange("p c k r -> p (c k) r"),
            in_=q16.rearrange("p c d -> p (c d)"),
            transpose=True,
        )
        hT4 = bigs.tile([P, NT, KD, P], F16, tag="hT4")
        nc.scalar.dma_start(
            out=hT4.rearrange("p c k r -> p (c k) r"),
            in_=h16.rearrange("p c d -> p (c d)"),
            transpose=True,
        )
        hw2T4 = bigs.tile([P, NT, KD, P], F16, tag="hw2T4")
        nc.scalar.dma_start(
            out=hw2T4.rearrange("p c k r -> p (c k) r"),
            in_=hw2.rearrange("p c d -> p (c d)"),
            transpose=True,
        )
        pT_all = bigs.tile([P, NT, L], F16, tag="pT", bufs=1, name="pT_all")
        bst.append(dict(
            b=b, q16=q16, h16=h16, qT4=qT4, hT4=hT4, hw2T4=hw2T4, pT=pT_all,
            m_negcol=smalls.tile([P, NT], F32, tag="m_negcol", name="m_negcol"),
            z_col=smalls.tile([P, NT], F32, tag="z_col", name="z_col"),
            r_col=smalls.tile([P, NT], F32, tag="r_col", name="r_col"),
        ))

    def emit_A_tile(s, i):
        # 512-col matmuls: the ISA max for fp32-PSUM output per instruction
        ps_a = psA.tile([P, L], F32, name="ps_a")
        for t0, t1 in ((0, 512), (512, 1024)):
            nc.tensor.matmul(ps_a[:, t0:t1], ones1, w1b16[:, t0:t1],
                             start=True, stop=False)
        # (h+w2)@qT pass first: its operands are ready before w1T4/hT4
        kds = range(0 if not SKIP_A_MM else KD, KD)
        for k in kds:
            st_hw2 = s["hw2T4"][:, i, k, :]
            for hf in range(2):
                nc.tensor.matmul(
                    ps_a[:, hf * 512:(hf + 1) * 512], st_hw2,
                    s["qT4"][:, hf * 4:(hf + 1) * 4, k, :],
                    start=False, stop=False,
                )
        for k in kds:
            st_h = s["hT4"][:, i, k, :]
            for hf in range(2):
                nc.tensor.matmul(
                    ps_a[:, hf * 512:(hf + 1) * 512], st_h,
                    w1T4[:, hf * 4:(hf + 1) * 4, k, :],
                    start=False, stop=(k == KD - 1),
                )
        if SKIP_A_MM:
            for t0, t1 in ((0, 512), (512, 1024)):
                nc.tensor.matmul(ps_a[:, t0:t1], ones1, w1b16[:, t0:t1],
                                 start=False, stop=True)
        negm = s["m_negcol"][:, i:i + 1]
        nc.vector.reduce_max(negm, ps_a, axis=AX, negate=True)
        p16 = scr.tile([P, L], F16, tag="p16", name="p16")
        nc.scalar.activation(out=p16, in_=ps_a, func=EXP, bias=negm,
                             scale=1.0, accum_out=s["z_col"][:, i:i + 1])
        nc.sync.dma_start(out=s["pT"][:, :, i * P:(i + 1) * P], in_=p16,
                          transpose=True)

    def emit_p2(s):
        # softmax over all 1024 row maxes via a 4KB DRAM-scratch rearrange
        m_true = smalls.tile([P, NT], F32, tag="m_true", name="m_true")
        nc.vector.tensor_sub(m_true, w2b_col, s["m_negcol"])
        m_dram = dram.tile([L], F32, tag="m_dram", name="m_dram")
        nc.sync.dma_start(out=m_dram.rearrange("(c p) -> p c", p=P), in_=m_true)
        m_row = smalls.tile([1, L], F32, tag="row_a", bufs=1, name="m_row")
        nc.sync.dma_start(out=m_row, in_=m_dram[None, :])
        negmm = smalls.tile([1, 1], F32, tag="negmm", name="negmm")
        nc.vector.reduce_max(negmm, m_row, axis=AX, negate=True)
        z2 = smalls.tile([1, 1], F32, tag="z2", name="z2")
        e2 = smalls.tile([1, L], F32, tag="e2", bufs=1, name="e2")
        nc.scalar.activation(out=e2, in_=m_row, func=EXP, bias=negmm,
                             scale=1.0, accum_out=z2)
        r2 = smalls.tile([1, 1], F32, tag="r2", name="r2")
        nc.vector.reciprocal(r2, z2)
        p2_row = smalls.tile([1, L], F32, tag="row_a", bufs=1, name="p2_row")
        nc.vector.tensor_scalar_mul(p2_row, in0=e2, scalar1=r2)
        p2_dram = dram.tile([L], F32, tag="p2_dram", name="p2_dram")
        nc.sync.dma_start(out=p2_dram[None, :], in_=p2_row)
        p2_col = smalls.tile([P, NT], F32, tag="p2_col", name="p2_col")
        nc.sync.dma_start(out=p2_col, in_=p2_dram.rearrange("(c p) -> p c", p=P))
        s["p2_col"] = p2_col

    def emit_B_tile(s, i):
        b, s0 = s["b"], i * P
        q16, h16 = s["q16"], s["h16"]
        ps_c = psC.tile([P, D], F32, name="ps_c")
        for tcn in range(NT):
            lp = s["pT"][:, tcn, s0:s0 + P]
            nc.tensor.matmul(ps_c[:, 0:512], lp, q16[:, tcn, 0:512],
                             start=(tcn == 0), stop=(tcn == NT - 1))
            nc.tensor.matmul(ps_c[:, 512:D], lp, q16[:, tcn, 512:D],
                             start=(tcn == 0), stop=(tcn == NT - 1))
        # sections 1-3 (c, h*c, qcc) assembled -> one 9KB-row DMA. The c
        # normalization runs on DVE (ACT AP-scale straight off PSUM gives
        # wrong results on hardware); the qcc scale runs on ACT from SBUF.
        osec = scr.tile([P, 3, D], F32, tag="osec", name="osec")
        r_i = s["r_col"][:, i:i + 1]
        # per-tile [128,1] reciprocal: multi-column DVE Reciprocal gives
        # wrong results on hardware (CoreSim accepts it)
        nc.vector.reciprocal(r_i, s["z_col"][:, i:i + 1])
        nc.vector.tensor_scalar_mul(osec[:, 0, :], in0=ps_c, scalar1=r_i)
        nc.vector.tensor_mul(osec[:, 1, :], h16[:, i, :], osec[:, 0, :])
        nc.scalar.activation(out=osec[:, 2, :], in_=osec[:, 1, :], func=COPY,
                             scale=s["p2_col"][:, i:i + 1])
        if not SKIP_OUT:
            nc.gpsimd.dma_start(out=out[b, s0:s0 + P, D:], in_=osec)

    # out section 0 is h verbatim: DRAM->DRAM copy, never touches SBUF
    if not SKIP_OUT:
        for b in range(BL):
            nc.sync.dma_start(out=out[b, :, 0:D], in_=h[b])

    # ---- schedule (safe): sequential per batch ----
    for b in range(BL):
        for i in range(NT):
            emit_A_tile(bst[b], i)
        emit_p2(bst[b])
        if not SKIP_B:
            for i in range(NT):
                emit_B_tile(bst[b], i)


def build():
    nc = bacc.Bacc()
    h = nc.dram_tensor("h", [BL, L, D], F32, kind="ExternalInput")
    q = nc.dram_tensor("q", [BL, L, D], F32, kind="ExternalInput")
    w1w = nc.dram_tensor("w1_w", [L, D], F32, kind="ExternalInput")
    w1b = nc.dram_tensor("w1_b", [L], F32, kind="ExternalInput")
    w2w = nc.dram_tensor("w2_w", [L, D], F32, kind="ExternalInput")
    w2b = nc.dram_tensor("w2_b", [L], F32, kind="ExternalInput")
    out = nc.dram_tensor("out", [BL, L, 4 * D], F32, kind="ExternalOutput")
    with tile.TileContext(nc) as tc, ExitStack() as ctx:
        _emit(ctx, tc, h[:], q[:], w1w[:], w1b[:], w2w[:], w2b[:], out[:])
    nc.compile()
    return nc


def _in_maps(inputs):
    arr = {k: np.ascontiguousarray(np.asarray(v, np.float32))
           for k, v in inputs.items()}
    maps = []
    for c in range(NCORES):
        sl = slice(c * BL, (c + 1) * BL)
        maps.append({
            "h": arr["h"][sl], "q": arr["q"][sl],
            "w1_w": arr["w1_w"], "w1_b": arr["w1_b"],
            "w2_w": arr["w2_w"], "w2_b": arr["w2_b"],
        })
    return maps


def kernel(**inputs):
    nc = build()
    res = run_bass_kernel_spmd(nc, _in_maps(inputs), core_ids=list(range(NCORES)))
    return np.concatenate([r["out"] for r in res.results], axis=0)


def run_profiled(inputs, **kwargs):
    nc = build()
    res = run_bass_kernel_spmd(
        nc, _in_maps(inputs), core_ids=list(range(NCORES)), trace=True, **kwargs
    )
    out = np.concatenate([r["out"] for r in res.results], axis=0)
    return out, res
